# revision 2
# baseline (speedup 1.0000x reference)
"""Euclidean distance layer (retrieval kNN) on 8 Trainium2 NeuronCores, v2.

out[b, o] = || x[b, :] - weight[:, o] ||_2   for x [2048, 1024], weight [1024, 16384].

Sharding: output columns across the 8 cores (2048 each).
Per core, d2 = x2[b] + w2[o] - 2 x@w_shard, out = sqrt(d2).

v2 vs baseline: no per-group PSUM seed matmuls and no shift-encode epilogue.
  - host passes xt = -sqrt(2)*x.T and w = sqrt(2)*w_shard in fp8e4, so the
    DoubleRow matmuls accumulate psum = -2*x@w directly
  - w2 = colsum((sqrt2 w)^2)/2 via 32 bf16 matmuls against a +0.5-constant
    stationary (reduction + partition-broadcast in one op); the [128, 512]
    broadcast tiles are copied to SBUF once and reused by every epilogue
  - x2 = rowsum(x^2) on DVE from an f16 copy of x
  - epilogue per [128, 512] tile: DVE scalar_tensor_tensor
    t = (psum + x2[p]) + w2b[j]  (both adds in one op), then ACT sqrt -> f16
PE work per body: 256 DoubleRow data matmuls + 32 bf16 w2 matmuls; everything
else rides on DVE/Pool/ACT under the PE critical path.
"""
import numpy as np

import concourse.bass as bass
import concourse.tile as tile
from concourse import bacc, mybir
from concourse.bass_utils import run_bass_kernel_spmd

f32 = mybir.dt.float32
f16 = mybir.dt.float16
bf16 = mybir.dt.bfloat16
fp8 = mybir.dt.float8e4
AF = mybir.ActivationFunctionType
ALU = mybir.AluOpType
DR = mybir.MatmulPerfMode.DoubleRow

B = 2048      # batch rows
I = 1024      # input size (contraction)
O = 16384     # output size (prototype count)
N_CORES = 8
OS = O // N_CORES   # 2048 output columns per core
P = 128       # partitions
NB = 512      # moving free-dim per matmul / psum bank
KT = I // P   # 8 k-tiles
MT = B // P   # 16 m-tiles
NT = OS // NB  # 4 n-blocks


def _emit_body(nc, tc, pools, c64, x_d, xt_d, w_d, out_d):
    if True:
        (xt_p, w_p, xr_p, wsq_p, w2b_p, x2_p, sq_p, t_p, o_p, ps_p,
         ps1_p) = pools

        xt_sb = xt_p.tile([P, KT, B], fp8)      # -sqrt2 * x.T, matmul stationary
        w_sb = w_p.tile([P, KT, OS], fp8)       # sqrt2 * w shard, matmul moving
        xr_sb = xr_p.tile([P, MT, I], f16)      # x rows (f16) for x2
        w2b = w2b_p.tile([P, NT, NB], f32)      # +w2 broadcast to all partitions
        x2col = x2_p.tile([P, MT], f32)         # x2 per-partition, one col per m

        xt_src = xt_d.ap().rearrange("(k p) b -> p k b", p=P)    # [128, KT, B]
        w_src = w_d.ap().rearrange("(k p) o -> p k o", p=P)      # [128, KT, OS]
        x_src = x_d.ap().rearrange("(m p) i -> p m i", p=P)      # [128, MT, I]

        # input DMAs, ordered so the PE's earliest dependencies land first:
        # w chunk n feeds squares -> w2 matmuls interleaved with m0; xt block
        # m0/m1 feeds the first data matmuls; x rows feed x2 (epilogues only).
        def dma_w(n):
            nc.sync.dma_start(w_sb[:, :, n * NB:(n + 1) * NB],
                              w_src[:, :, n * NB:(n + 1) * NB])

        def dma_x(c):
            nc.sync.dma_start(xr_sb[:, c * 4:(c + 1) * 4, :],
                              x_src[:, c * 4:(c + 1) * 4, :])

        dma_w(0)
        dma_w(1)
        nc.sync.dma_start(xt_sb[:, :, 0:2 * P], xt_src[:, :, 0:2 * P])
        dma_x(0)
        dma_w(2)
        dma_w(3)
        for c in range(1, 4):
            dma_x(c)
        nc.sync.dma_start(xt_sb[:, :, 2 * P:8 * P], xt_src[:, :, 2 * P:8 * P])
        nc.sync.dma_start(xt_sb[:, :, 8 * P:], xt_src[:, :, 8 * P:])

        # x2 rowsums, split DVE/ACT by m-parity to balance both engines under
        # the PE; emitted in m-chunks so later queue work interleaves
        def emit_x2(m0, m1):
            for m in range(m0, m1):
                sq = sq_p.tile([P, I], f16, name=f"sq{m % 2}")
                if m % 2 == 0:
                    nc.vector.scalar_tensor_tensor(
                        sq[:], xr_sb[:, m, :], 1.0, xr_sb[:, m, :],
                        op0=ALU.mult, op1=ALU.mult,
                        accum_out=x2col[:, m:m + 1])
                else:
                    nc.scalar.activation(sq[:], xr_sb[:, m, :], AF.Square,
                                         accum_out=x2col[:, m:m + 1])

        pss = {}

        def mm_block(m, nlist, start=True):
            msl = slice(m * P, (m + 1) * P)
            for kp in range(KT // 2):
                for n in nlist:
                    nc.tensor.matmul(
                        pss[m, n][:],
                        xt_sb[:, 2 * kp:2 * kp + 2, msl],
                        w_sb[:, 2 * kp:2 * kp + 2, n * NB:(n + 1) * NB],
                        start=(kp == 0 and start),
                        stop=(kp == KT // 2 - 1),
                        perf_mode=DR, skip_group_check=True)

        def ps_alloc(m, nlist):
            for n in nlist:
                pool = ps1_p if n == 3 else ps_p
                pss[m, n] = pool.tile([P, NB], f32, name=f"ps{n}")

        def out_dma(m, osb):
            # output rides the Pool DGE queue so it never blocks the next
            # body's input DMAs on the SP queue
            nc.gpsimd.dma_start(out_d.ap()[m * P:(m + 1) * P, :], osb[:])

        S_ACT = 5.656854249492381   # (4*sqrt2*w')^2 = 32*w'^2 = 64*w^2

        def w2_group(n):
            # squares wsq = 64*w^2 as fp8 DoubleRow pairs, split ACT/DVE so
            # neither chain gates the w2 matmuls; then
            # psw = sum_k (1/64)*64*w^2 = w2, broadcast across partitions by
            # the constant stationary (DoubleRow fp8 like the data matmuls)
            ns = slice(n * NB, (n + 1) * NB)
            wsqs = []
            for j in range(KT // 2):
                wsq = wsq_p.tile([P, 2, NB], fp8, name=f"wsq{n}{j}")
                src = w_sb[:, 2 * j:2 * j + 2, ns]
                if j < 2:
                    nc.scalar.activation(wsq[:], src, AF.Square, scale=S_ACT)
                else:
                    nc.vector.scalar_tensor_tensor(
                        wsq[:], src, 32.0, src, op0=ALU.mult, op1=ALU.mult)
                wsqs.append(wsq)
            psw = ps1_p.tile([P, NB], f32, name="psw")
            for j in range(KT // 2):
                nc.tensor.matmul(psw[:], c64[:], wsqs[j][:],
                                 start=(j == 0), stop=(j == KT // 2 - 1),
                                 perf_mode=DR, skip_group_check=True)
            nc.scalar.activation(w2b[:, n], psw[:], AF.Copy)

        def epilogue_stt(m):
            # DVE adds both rank-1 terms in one op, ACT takes the root
            osb = o_p.tile([P, OS], f16)
            for n in range(NT):
                ns = slice(n * NB, (n + 1) * NB)
                t = t_p.tile([P, NB], f32)
                nc.vector.scalar_tensor_tensor(
                    t[:], pss.pop((m, n))[:], x2col[:, m:m + 1], w2b[:, n],
                    op0=ALU.add, op1=ALU.add)
                nc.scalar.activation(osb[:, ns], t[:], AF.Sqrt)
            out_dma(m, osb)

        # m0 interleaves with the w2 groups chunk by chunk as w lands, so the
        # PE ramps on data work while the squares chains complete. m1's n3
        # group trails epilogue_stt(0) because PSUM holds only 8 banks
        # (ps0-2 double-buffered, ps3, psw).
        ps_alloc(0, range(NT))
        for n in range(NT):
            mm_block(0, [n])
            w2_group(n)
        emit_x2(0, 4)
        ps_alloc(1, range(NT - 1))
        mm_block(1, range(NT - 1))
        epilogue_stt(0)
        ps_alloc(1, [NT - 1])
        mm_block(1, [NT - 1])
        epilogue_stt(1)

        for m in range(2, MT):
            if m in (4, 8, 12):
                emit_x2(m, m + 4)
            ps_alloc(m, range(NT))
            mm_block(m, range(NT))
            epilogue_stt(m)


def build(repeats=1):
    from contextlib import ExitStack
    nc = bacc.Bacc("TRN2", target_bir_lowering=False, debug=False,
                   num_devices=N_CORES)
    x_d = nc.dram_tensor("x", [B, I], f16, kind="ExternalInput")
    xt_d = nc.dram_tensor("xt", [I, B], fp8, kind="ExternalInput")
    w_d = nc.dram_tensor("w", [I, OS], fp8, kind="ExternalInput")
    out_d = nc.dram_tensor("out", [B, OS], f16, kind="ExternalOutput")
    with tile.TileContext(nc) as tc, ExitStack() as ctx:
        const_p = ctx.enter_context(tc.tile_pool(name="const", bufs=1))
        pools = (
            ctx.enter_context(tc.tile_pool(name="xt", bufs=2)),
            ctx.enter_context(tc.tile_pool(name="w", bufs=2)),
            ctx.enter_context(tc.tile_pool(name="xr", bufs=2)),
            ctx.enter_context(tc.tile_pool(name="wsq", bufs=1)),
            ctx.enter_context(tc.tile_pool(name="w2b", bufs=2)),
            ctx.enter_context(tc.tile_pool(name="x2", bufs=2)),
            ctx.enter_context(tc.tile_pool(name="sq", bufs=2)),
            ctx.enter_context(tc.tile_pool(name="t", bufs=4)),
            ctx.enter_context(tc.tile_pool(name="o", bufs=3)),
            ctx.enter_context(tc.tile_pool(name="ps", bufs=2, space="PSUM")),
            ctx.enter_context(tc.tile_pool(name="ps1", bufs=1, space="PSUM")),
        )
        c64 = const_p.tile([P, 2, P], fp8)
        nc.vector.memset(c64[:], 1.0 / 64.0)
        # touch Square and Sqrt once so any ACT table loads happen at t=0,
        # under the DMA lead-in, instead of on the first real tile
        warm = const_p.tile([P, 1], f32)
        nc.scalar.activation(warm[:], c64[:, 0, 0:1], AF.Square)
        nc.scalar.activation(warm[:], c64[:, 0, 0:1], AF.Sqrt)
        for _ in range(repeats):
            _emit_body(nc, tc, pools, c64, x_d, xt_d, w_d, out_d)
    nc.compile()
    return nc


_NC = None


def make_in_maps(x, weight):
    import ml_dtypes
    s = np.float32(np.sqrt(2.0))
    x16 = np.ascontiguousarray(x.astype(np.float16))
    xt = np.ascontiguousarray((-s * x.T).astype(ml_dtypes.float8_e4m3))
    return [{"x": x16, "xt": xt,
             "w": np.ascontiguousarray(
                 (s * weight[:, c * OS:(c + 1) * OS]).astype(ml_dtypes.float8_e4m3))}
            for c in range(N_CORES)]


def assemble(results):
    return np.ascontiguousarray(np.concatenate(
        [r["out"].astype(np.float32) for r in results], axis=1))


def kernel(x, weight):
    global _NC
    x = np.asarray(x, dtype=np.float32)
    weight = np.asarray(weight, dtype=np.float32)
    if _NC is None:
        _NC = build(repeats=1)
    in_maps = make_in_maps(x, weight)
    res = run_bass_kernel_spmd(_NC, in_maps, core_ids=list(range(N_CORES)))
    return assemble(res.results)


# revision 3
# speedup vs baseline: 1.0553x; 1.0553x over previous
"""Euclidean distance layer (retrieval kNN) on 8 Trainium2 NeuronCores.

out[b, o] = || x[b, :] - weight[:, o] ||_2   for x [2048, 1024], weight [1024, 16384].

Sharding: output columns across the 8 cores (2048 each).
Per core, d2 = x2[b] + w2[o] - 2 x@w_shard, out = sqrt(d2).

vs the seeded baseline: no per-group PSUM seed matmuls, no shift-encode.
  - host passes xt = -sqrt(2)*x.T and w = sqrt(2)*w_shard in fp8e4, so the
    fp8 DoubleRow matmuls accumulate psum = -2*x@w directly (256 MMs/body,
    ~107 ns each on hw — the fp8-DR roofline)
  - w2 = colsum(w^2) via scaled-fp8 squares (ACT Square scale=4sqrt2 and DVE
    stt (32w)*w, split so neither chain gates) into 16 DoubleRow matmuls
    against a 1/64-constant stationary: reduction + partition-broadcast in
    one op; the [128, 512] broadcast tiles land in SBUF via ACT copies
  - x2 = rowsum(x^2) from an f16 copy of x, split DVE/ACT by m-parity
  - epilogue per [128, 512] tile: DVE scalar_tensor_tensor
    t = (psum + x2[p]) + w2b[j]  (both adds in one op), then ACT sqrt -> f16
  - DMA queue discipline is critical on this machine: inputs on the SP HWDGE
    queue, outputs on the gpsimd SWDGE queue; issuing DMAs from the ACT
    queue stalls ACT compute, and a single shared queue serializes bodies
PE does 272 DoubleRow matmuls/body; DVE/ACT each carry about half the
epilogue + norm work and stay under the PE critical path.
"""
import numpy as np

import concourse.bass as bass
import concourse.tile as tile
from concourse import bacc, mybir
from concourse.bass_utils import run_bass_kernel_spmd

f32 = mybir.dt.float32
f16 = mybir.dt.float16
bf16 = mybir.dt.bfloat16
fp8 = mybir.dt.float8e4
AF = mybir.ActivationFunctionType
ALU = mybir.AluOpType
DR = mybir.MatmulPerfMode.DoubleRow

B = 2048      # batch rows
I = 1024      # input size (contraction)
O = 16384     # output size (prototype count)
N_CORES = 8
OS = O // N_CORES   # 2048 output columns per core
P = 128       # partitions
NB = 512      # moving free-dim per matmul / psum bank
KT = I // P   # 8 k-tiles
MT = B // P   # 16 m-tiles
NT = OS // NB  # 4 n-blocks


def _emit_body(nc, tc, pools, c64, x_d, xt_d, w_d, out_d):
    if True:
        (xt_p, w_p, xr_p, wsq_p, w2b_p, x2_p, sq_p, t_p, o_p, ps_p,
         ps1_p) = pools

        xt_sb = xt_p.tile([P, KT, B], fp8)      # -sqrt2 * x.T, matmul stationary
        w_sb = w_p.tile([P, KT, OS], fp8)       # sqrt2 * w shard, matmul moving
        xr_sb = xr_p.tile([P, MT, I], f16)      # x rows (f16) for x2
        w2b = w2b_p.tile([P, NT, NB], f32)      # +w2 broadcast to all partitions
        x2col = x2_p.tile([P, MT], f32)         # x2 per-partition, one col per m

        xt_src = xt_d.ap().rearrange("(k p) b -> p k b", p=P)    # [128, KT, B]
        w_src = w_d.ap().rearrange("(k p) o -> p k o", p=P)      # [128, KT, OS]
        x_src = x_d.ap().rearrange("(m p) i -> p m i", p=P)      # [128, MT, I]

        # input DMAs, ordered so the PE's earliest dependencies land first:
        # w chunk n feeds squares -> w2 matmuls interleaved with m0; xt block
        # m0/m1 feeds the first data matmuls; x rows feed x2 (epilogues only).
        def dma_w(n):
            nc.sync.dma_start(w_sb[:, :, n * NB:(n + 1) * NB],
                              w_src[:, :, n * NB:(n + 1) * NB])

        def dma_x(c):
            nc.sync.dma_start(xr_sb[:, c * 4:(c + 1) * 4, :],
                              x_src[:, c * 4:(c + 1) * 4, :])

        dma_w(0)
        dma_w(1)
        nc.sync.dma_start(xt_sb[:, :, 0:2 * P], xt_src[:, :, 0:2 * P])
        dma_x(0)
        dma_w(2)
        dma_w(3)
        for c in range(1, 4):
            dma_x(c)
        nc.sync.dma_start(xt_sb[:, :, 2 * P:8 * P], xt_src[:, :, 2 * P:8 * P])
        nc.sync.dma_start(xt_sb[:, :, 8 * P:], xt_src[:, :, 8 * P:])

        # x2 rowsums, split DVE/ACT by m-parity to balance both engines under
        # the PE; emitted in m-chunks so later queue work interleaves
        def emit_x2(m0, m1):
            for m in range(m0, m1):
                sq = sq_p.tile([P, I], f16, name=f"sq{m % 2}")
                if m % 2 == 0:
                    nc.vector.scalar_tensor_tensor(
                        sq[:], xr_sb[:, m, :], 1.0, xr_sb[:, m, :],
                        op0=ALU.mult, op1=ALU.mult,
                        accum_out=x2col[:, m:m + 1])
                else:
                    nc.scalar.activation(sq[:], xr_sb[:, m, :], AF.Square,
                                         accum_out=x2col[:, m:m + 1])

        pss = {}

        def mm_block(m, nlist, start=True):
            msl = slice(m * P, (m + 1) * P)
            for kp in range(KT // 2):
                for n in nlist:
                    nc.tensor.matmul(
                        pss[m, n][:],
                        xt_sb[:, 2 * kp:2 * kp + 2, msl],
                        w_sb[:, 2 * kp:2 * kp + 2, n * NB:(n + 1) * NB],
                        start=(kp == 0 and start),
                        stop=(kp == KT // 2 - 1),
                        perf_mode=DR, skip_group_check=True)

        def ps_alloc(m, nlist):
            for n in nlist:
                pool = ps1_p if n == 3 else ps_p
                pss[m, n] = pool.tile([P, NB], f32, name=f"ps{n}")

        def out_dma(m, osb):
            # output rides the Pool DGE queue so it never blocks the next
            # body's input DMAs on the SP queue
            nc.gpsimd.dma_start(out_d.ap()[m * P:(m + 1) * P, :], osb[:])

        S_ACT = 5.656854249492381   # (4*sqrt2*w')^2 = 32*w'^2 = 64*w^2

        def w2_group(n):
            # squares wsq = 64*w^2 as fp8 DoubleRow pairs, split ACT/DVE so
            # neither chain gates the w2 matmuls; then
            # psw = sum_k (1/64)*64*w^2 = w2, broadcast across partitions by
            # the constant stationary (DoubleRow fp8 like the data matmuls)
            ns = slice(n * NB, (n + 1) * NB)
            wsqs = []
            for j in range(KT // 2):
                wsq = wsq_p.tile([P, 2, NB], fp8, name=f"wsq{n}{j}")
                src = w_sb[:, 2 * j:2 * j + 2, ns]
                if j < 2:
                    nc.scalar.activation(wsq[:], src, AF.Square, scale=S_ACT)
                else:
                    nc.vector.scalar_tensor_tensor(
                        wsq[:], src, 32.0, src, op0=ALU.mult, op1=ALU.mult)
                wsqs.append(wsq)
            psw = ps1_p.tile([P, NB], f32, name="psw")
            for j in range(KT // 2):
                nc.tensor.matmul(psw[:], c64[:], wsqs[j][:],
                                 start=(j == 0), stop=(j == KT // 2 - 1),
                                 perf_mode=DR, skip_group_check=True)
            nc.scalar.activation(w2b[:, n], psw[:], AF.Copy)

        def epilogue_stt(m):
            # DVE adds both rank-1 terms in one op, ACT takes the root
            osb = o_p.tile([P, OS], f16)
            for n in range(NT):
                ns = slice(n * NB, (n + 1) * NB)
                t = t_p.tile([P, NB], f32)
                nc.vector.scalar_tensor_tensor(
                    t[:], pss.pop((m, n))[:], x2col[:, m:m + 1], w2b[:, n],
                    op0=ALU.add, op1=ALU.add)
                nc.scalar.activation(osb[:, ns], t[:], AF.Sqrt)
            out_dma(m, osb)

        # m0 interleaves with the w2 groups chunk by chunk as w lands, so the
        # PE ramps on data work while the squares chains complete. m1's n3
        # group trails epilogue_stt(0) because PSUM holds only 8 banks
        # (ps0-2 double-buffered, ps3, psw).
        ps_alloc(0, range(NT))
        for n in range(NT):
            mm_block(0, [n])
            w2_group(n)
        emit_x2(0, 4)
        ps_alloc(1, range(NT - 1))
        mm_block(1, range(NT - 1))
        epilogue_stt(0)
        ps_alloc(1, [NT - 1])
        mm_block(1, [NT - 1])
        epilogue_stt(1)

        for m in range(2, MT):
            if m in (4, 8, 12):
                emit_x2(m, m + 4)
            ps_alloc(m, range(NT))
            mm_block(m, range(NT))
            epilogue_stt(m)


def build(repeats=1):
    from contextlib import ExitStack
    nc = bacc.Bacc("TRN2", target_bir_lowering=False, debug=False,
                   num_devices=N_CORES)
    x_d = nc.dram_tensor("x", [B, I], f16, kind="ExternalInput")
    xt_d = nc.dram_tensor("xt", [I, B], fp8, kind="ExternalInput")
    w_d = nc.dram_tensor("w", [I, OS], fp8, kind="ExternalInput")
    out_d = nc.dram_tensor("out", [B, OS], f16, kind="ExternalOutput")
    with tile.TileContext(nc) as tc, ExitStack() as ctx:
        const_p = ctx.enter_context(tc.tile_pool(name="const", bufs=1))
        pools = (
            ctx.enter_context(tc.tile_pool(name="xt", bufs=2)),
            ctx.enter_context(tc.tile_pool(name="w", bufs=2)),
            ctx.enter_context(tc.tile_pool(name="xr", bufs=2)),
            ctx.enter_context(tc.tile_pool(name="wsq", bufs=1)),
            ctx.enter_context(tc.tile_pool(name="w2b", bufs=2)),
            ctx.enter_context(tc.tile_pool(name="x2", bufs=2)),
            ctx.enter_context(tc.tile_pool(name="sq", bufs=2)),
            ctx.enter_context(tc.tile_pool(name="t", bufs=4)),
            ctx.enter_context(tc.tile_pool(name="o", bufs=3)),
            ctx.enter_context(tc.tile_pool(name="ps", bufs=2, space="PSUM")),
            ctx.enter_context(tc.tile_pool(name="ps1", bufs=1, space="PSUM")),
        )
        c64 = const_p.tile([P, 2, P], fp8)
        nc.vector.memset(c64[:], 1.0 / 64.0)
        # touch Square and Sqrt once so any ACT table loads happen at t=0,
        # under the DMA lead-in, instead of on the first real tile
        warm = const_p.tile([P, 1], f32)
        nc.scalar.activation(warm[:], c64[:, 0, 0:1], AF.Square)
        nc.scalar.activation(warm[:], c64[:, 0, 0:1], AF.Sqrt)
        for _ in range(repeats):
            _emit_body(nc, tc, pools, c64, x_d, xt_d, w_d, out_d)
    nc.compile()
    return nc


_NC = None


def make_in_maps(x, weight):
    import ml_dtypes
    s = np.float32(np.sqrt(2.0))
    x16 = np.ascontiguousarray(x.astype(np.float16))
    xt = np.ascontiguousarray((-s * x.T).astype(ml_dtypes.float8_e4m3))
    return [{"x": x16, "xt": xt,
             "w": np.ascontiguousarray(
                 (s * weight[:, c * OS:(c + 1) * OS]).astype(ml_dtypes.float8_e4m3))}
            for c in range(N_CORES)]


def assemble(results):
    return np.ascontiguousarray(np.concatenate(
        [r["out"].astype(np.float32) for r in results], axis=1))


def kernel(x, weight):
    global _NC
    x = np.asarray(x, dtype=np.float32)
    weight = np.asarray(weight, dtype=np.float32)
    if _NC is None:
        _NC = build(repeats=1)
    in_maps = make_in_maps(x, weight)
    res = run_bass_kernel_spmd(_NC, in_maps, core_ids=list(range(N_CORES)))
    return assemble(res.results)
